# revision 14
# baseline (speedup 1.0000x reference)
"""PointPillarScatter (intersweep, 3 bins) Trainium2 Bass kernel.

Problem: for each of 3 bins, scatter 64000 pillar rows [64 feats] into a
[B=4, C=64, NY=496, NX=432] canvas at (b, :, y, x); empty cells are zero.

Strategy (8 NeuronCores, SPMD):
  - The output is 658 MB, ~92.5% zeros -> write-bandwidth bound. Dense
    output tiles (zeros included) are generated on-chip and stored with
    large contiguous DMAs; the compute stages are sized to fit under the
    ~19 us per-chunk out-DMA.
  - Host-side (cheap numpy): shard the 12 (bin, b) canvases into 48
    quarter-canvases of 124 y-rows; 6 per core, processed as 3 pairs.
    A window is ONE canvas row (432 cells); a row's pillars (max 36) go
    into RP=40 padded slots.
  - Per window one fp16 matmul places the pillar features:
      out[128, 432] = lhsT[128, 128].T @ onehot[128, 432]
    K rows hold [2 quarters x 40 slots] (A at 0:40, B at 64:104); all
    other K rows stay zero so the matmul runs with K=128 (full PE rate;
    K<128 runs half-rate). Features ship as a single fp16 term
    (rel err ~3e-4, well under the 2e-2 gate) -- a third the traffic of
    an exact 3-term bf16 split.
    onehot[k, c] = (x[k] == c): one DVE tensor_scalar is_equal of an
    fp16 iota row against per-partition relc scalars (432 < 2048 so
    fp16 holds the indices exactly; bf16 would not past 256).
  - lhsT is block-"diagonal" over quarters (A in M-cols 0:64, B in
    64:128). The compact feature slab is DMA'd contiguously (one ~4KB
    descriptor per partition; a strided DMA into lt shatters into 128B
    descriptors) and placed into the persistent pre-zeroed lt tiles by
    one GpSimd + one DVE block copy (partition starts must be 32-aligned).
  - 4 windows share one 4-bank PSUM tile; one wide ACT copy moves
    [128, 4, 432] to SBUF staging; one [128 x 53568 B] contiguous DMA
    per 31-window chunk writes out. Host de-interleaves the quarters.
  - in-DMAs ride the GpSimd SWDGE ring: they never queue behind the
    ~16 us out-DMAs on the sync-engine HWDGE ring (FIFO per ring), and
    their descriptor generation stays off the busy ACT engine.
"""

import numpy as np

import concourse.bass as bass
import concourse.tile as tile
from concourse import bacc, mybir
from concourse.bass_utils import run_bass_kernel_spmd

# Problem geometry (hardcoded; kernel.py must be self-contained).
B = 4
C = 64
NX = 432
NY = 496
NBINS = 3
NCORES = 8

NQ = NBINS * B * 4  # 48 quarter-canvases
YQ = NY // 4  # 124 y-rows per quarter
QPC = NQ // NCORES  # 6 quarters per core
PAIRS = QPC // 2  # 3 pairs per core
NW = NX  # one canvas row per window
WPQ = YQ  # 124 windows per quarter-pair
W = PAIRS * WPQ  # 372 windows per core
CH = 31  # windows per staging chunk
NCHUNKS = WPQ // CH  # 4 chunks per pair
GRP = 4  # windows per PSUM tile / ACT copy
RP = 40  # padded pillar slots per window per quarter (max count is 36)

_cache = {}


def _build():
    nc = bacc.Bacc(trn_type="TRN2")
    f16 = mybir.dt.float16
    f32 = mybir.dt.float32
    lhst_d = nc.dram_tensor("lhst", [2 * RP, W, C], f16,
                            kind="ExternalInput")
    iota_d = nc.dram_tensor("iotat", [128, NW], f16, kind="ExternalInput")
    relc_d = nc.dram_tensor("relc", [128, W], f32, kind="ExternalInput")
    out_d = nc.dram_tensor("out", [PAIRS, NCHUNKS, 128, CH, NW], f32,
                           kind="ExternalOutput")

    with tile.TileContext(nc) as tc:
        with (
            tc.tile_pool(name="const", bufs=1) as constp,
            tc.tile_pool(name="cpt", bufs=2) as cptp,
            tc.tile_pool(name="lhstp", bufs=1) as lhstp,
            tc.tile_pool(name="maskp", bufs=6) as maskp,
            tc.tile_pool(name="stage", bufs=3) as stagep,
            tc.tile_pool(name="psum", bufs=2, space=bass.MemorySpace.PSUM) as psump,
        ):
            iota = constp.tile([128, NW], f16, name="iota")
            relc = constp.tile([128, W], f32, name="relc")
            nc.scalar.dma_start(out=iota[:], in_=iota_d[:])
            nc.scalar.dma_start(out=relc[:], in_=relc_d[:])
            # persistent ping-pong stationary tiles; zero regions (pad rows,
            # off-diagonal col blocks) are memset once and never rewritten.
            # lt1's memset is deferred into chunk 0's slot so the startup
            # chain to the first matmul stays short.
            lts = [lhstp.tile([128, CH, 128], f16, name=f"lt{pp}",
                              tag=f"lt{pp}") for pp in range(2)]
            nc.vector.memset(lts[0][:], 0.0)
            for pair in range(PAIRS):
                for ch in range(NCHUNKS):
                    g0 = pair * WPQ + ch * CH
                    ci = pair * NCHUNKS + ch
                    lt = lts[ci % 2]
                    # contiguous compact load (one ~4KB descriptor per
                    # partition; a strided DMA into lt shatters into 128B
                    # descriptors), then block placement copies into the
                    # 32-aligned partition starts 0 / 64
                    cpt = cptp.tile([64 + RP, CH, C], f16, name="cpt")
                    nc.scalar.dma_start(out=cpt[0:RP],
                                        in_=lhst_d[0:RP, g0:g0 + CH, :])
                    nc.scalar.dma_start(out=cpt[64:64 + RP],
                                        in_=lhst_d[RP:2 * RP, g0:g0 + CH, :])
                    if ci == 0:
                        # fast DVE copies for chunk 0: first matmul can
                        # start ~7us in instead of waiting a 7us Q7 copy
                        nc.vector.tensor_copy(out=lt[0:RP, :, 0:C],
                                              in_=cpt[0:RP])
                    else:
                        nc.gpsimd.tensor_copy(out=lt[0:RP, :, 0:C],
                                              in_=cpt[0:RP])
                    nc.vector.tensor_copy(out=lt[64:64 + RP, :, C:128],
                                          in_=cpt[64:64 + RP])
                    if ci == 0:
                        nc.gpsimd.memset(lts[1][:], 0.0)
                    st = stagep.tile([128, CH, NW], f32, name="st")
                    sub0 = 0
                    ngrp = (CH + GRP - 1) // GRP
                    for grp in range(ngrp):
                        j0 = grp * GRP
                        n = min(GRP, CH - j0)
                        acc = psump.tile([128, GRP, 512], f32, name="acc")
                        for par in range(n):
                            w = j0 + par
                            mask = maskp.tile([128, NW], f16, name="mask")
                            nc.vector.tensor_scalar(
                                out=mask[:],
                                in0=iota[:],
                                scalar1=relc[:, g0 + w:g0 + w + 1],
                                scalar2=None,
                                op0=mybir.AluOpType.is_equal,
                            )
                            nc.tensor.matmul(acc[:, par, 0:NW], lt[:, w, :],
                                             mask[:], start=True, stop=True)
                        nc.scalar.copy(st[:, j0:j0 + n, :], acc[:, 0:n, 0:NW])
                        # chunk 0 only: sub-chunk stores so the out queue
                        # starts ~8 windows in; later chunks use one big
                        # store (53KB descriptors run at the 27GB/s
                        # per-engine port cap; quarter-size ones pay ~12%
                        # more engine-busy)
                        if pair == 0 and ch == 0 and (grp % 2 == 1
                                                      or grp == ngrp - 1):
                            hi = j0 + n
                            nc.sync.dma_start(
                                out=out_d[pair, ch, :, sub0:hi, :],
                                in_=st[:, sub0:hi, :])
                            sub0 = hi
                    if not (pair == 0 and ch == 0):
                        nc.sync.dma_start(out=out_d[pair, ch], in_=st[:])
    nc.compile()
    return nc


def _pack(inputs):
    lhst = np.zeros((NCORES, 2 * RP, W, C), np.float16)
    iota = np.broadcast_to(np.arange(NW, dtype=np.float16),
                           (128, NW)).copy()
    relc_a = np.full((NCORES, 128, W), -1.0, np.float32)

    for bin_i in range(NBINS):
        feats = np.asarray(inputs[f"pillar_features_bin_{bin_i}"],
                           np.float32).astype(np.float16)
        coords = np.asarray(inputs[f"voxel_coords_bin_{bin_i}"])
        cb = np.asarray(coords[:, 0], np.int64)
        cy = np.asarray(coords[:, 2], np.int64)
        cx = np.asarray(coords[:, 3], np.int64)
        for b in range(B):
            rows_b = np.nonzero(cb == b)[0]
            y_b, x_b = cy[rows_b], cx[rows_b]
            for yq in range(4):
                q = bin_i * 16 + b * 4 + yq
                core, j = divmod(q, QPC)
                pair, half = divmod(j, 2)
                sel = (y_b >= YQ * yq) & (y_b < YQ * (yq + 1))
                rows = rows_b[sel]
                wv = y_b[sel] - YQ * yq
                rel = x_b[sel]
                order = np.argsort(wv, kind="stable")
                rows, wv, rel = rows[order], wv[order], rel[order]
                cnt = np.bincount(wv, minlength=WPQ)
                if cnt.max() > RP:
                    raise OverflowError(int(cnt.max()))
                off = np.concatenate([[0], np.cumsum(cnt)[:-1]])
                slot = np.arange(len(rows)) - off[wv]
                wins = pair * WPQ + wv
                lhst[core, half * RP + slot, wins, :] = feats[rows]
                relc_a[core, half * 64 + slot, wins] = rel
    return [{"lhst": lhst[c], "iotat": iota, "relc": relc_a[c]}
            for c in range(NCORES)]


def _run(inputs, trace=False):
    if "nc" not in _cache:
        _cache["nc"] = _build()
    nc = _cache["nc"]
    in_maps = _pack(inputs)
    res = run_bass_kernel_spmd(nc, in_maps, core_ids=list(range(NCORES)),
                               trace=trace)
    outs = [np.zeros((B, C, NY, NX), np.float32) for _ in range(NBINS)]
    for q in range(NQ):
        bin_i, rem = divmod(q, 16)
        b, yq = divmod(rem, 4)
        core, j = divmod(q, QPC)
        pair, half = divmod(j, 2)
        # [NCHUNKS, 64, CH, NW] -> [64, QCELLS] -> [64, YQ, NX]
        blk = res.results[core]["out"][pair, :, half * C:(half + 1) * C]
        outs[bin_i][b, :, YQ * yq:YQ * (yq + 1), :] = (
            blk.transpose(1, 0, 2, 3).reshape(C, YQ, NX))
    return tuple(outs), res


def kernel(**inputs):
    out, _ = _run(inputs)
    return out


def kernel_traced(**inputs):
    """Like kernel() but also returns BassKernelResults (for test.py)."""
    return _run(inputs, trace=True)


# revision 15
# speedup vs baseline: 1.0467x; 1.0467x over previous
"""PointPillarScatter (intersweep, 3 bins) Trainium2 Bass kernel.

Problem: for each of 3 bins, scatter 64000 pillar rows [64 feats] into a
[B=4, C=64, NY=496, NX=432] canvas at (b, :, y, x); empty cells are zero.

Strategy (8 NeuronCores, SPMD):
  - The output is 658 MB, ~92.5% zeros -> write-bandwidth bound. Dense
    output tiles (zeros included) are generated on-chip and stored with
    large contiguous DMAs; the compute stages are sized to fit under the
    ~19 us per-chunk out-DMA.
  - Host-side (cheap numpy): shard the 12 (bin, b) canvases into 48
    quarter-canvases of 124 y-rows; 6 per core, processed as 3 pairs.
    A window is ONE canvas row (432 cells); a row's pillars (max 36) go
    into RP=40 padded slots.
  - Per window one fp16 matmul places the pillar features:
      out[128, 432] = lhsT[128, 128].T @ onehot[128, 432]
    K rows hold [2 quarters x 40 slots] (A at 0:40, B at 64:104); all
    other K rows stay zero so the matmul runs with K=128 (full PE rate;
    K<128 runs half-rate). Features ship as a single fp16 term
    (rel err ~3e-4, well under the 2e-2 gate) -- a third the traffic of
    an exact 3-term bf16 split.
    onehot[k, c] = (x[k] == c): one DVE tensor_scalar is_equal of an
    fp16 iota row against per-partition relc scalars (432 < 2048 so
    fp16 holds the indices exactly; bf16 would not past 256).
  - lhsT is block-"diagonal" over quarters (A in M-cols 0:64, B in
    64:128). The compact feature slab is DMA'd contiguously (one ~4KB
    descriptor per partition; a strided DMA into lt shatters into 128B
    descriptors) and placed into the persistent pre-zeroed lt tiles by
    one GpSimd + one DVE block copy (partition starts must be 32-aligned).
  - 4 windows share one 4-bank PSUM tile; one wide ACT copy moves
    [128, 4, 432] to SBUF staging; one [128 x 53568 B] contiguous DMA
    per 31-window chunk writes out. Host de-interleaves the quarters.
  - in-DMAs ride the GpSimd SWDGE ring: they never queue behind the
    ~16 us out-DMAs on the sync-engine HWDGE ring (FIFO per ring), and
    their descriptor generation stays off the busy ACT engine.
"""

import numpy as np

import concourse.bass as bass
import concourse.tile as tile
from concourse import bacc, mybir
from concourse.bass_utils import run_bass_kernel_spmd

# Problem geometry (hardcoded; kernel.py must be self-contained).
B = 4
C = 64
NX = 432
NY = 496
NBINS = 3
NCORES = 8

NQ = NBINS * B * 4  # 48 quarter-canvases
YQ = NY // 4  # 124 y-rows per quarter
QPC = NQ // NCORES  # 6 quarters per core
PAIRS = QPC // 2  # 3 pairs per core
NW = NX  # one canvas row per window
WPQ = YQ  # 124 windows per quarter-pair
W = PAIRS * WPQ  # 372 windows per core
CH = 31  # windows per staging chunk
NCHUNKS = WPQ // CH  # 4 chunks per pair
GRP = 4  # windows per PSUM tile / ACT copy
RP = 40  # padded pillar slots per window per quarter (max count is 36)

_cache = {}


def _build():
    nc = bacc.Bacc(trn_type="TRN2")
    f16 = mybir.dt.float16
    f32 = mybir.dt.float32
    lhst_d = nc.dram_tensor("lhst", [2 * RP, W, C], f16,
                            kind="ExternalInput")
    iota_d = nc.dram_tensor("iotat", [128, NW], f16, kind="ExternalInput")
    relc_d = nc.dram_tensor("relc", [128, W], f32, kind="ExternalInput")
    out_d = nc.dram_tensor("out", [PAIRS, NCHUNKS, 128, CH, NW], f32,
                           kind="ExternalOutput")

    with tile.TileContext(nc) as tc:
        with (
            tc.tile_pool(name="const", bufs=1) as constp,
            tc.tile_pool(name="lhstp", bufs=1) as lhstp,
            tc.tile_pool(name="maskp", bufs=6) as maskp,
            tc.tile_pool(name="stage", bufs=2) as stagep,
            tc.tile_pool(name="psum", bufs=2, space=bass.MemorySpace.PSUM) as psump,
        ):
            iota = constp.tile([128, NW], f16, name="iota")
            relc = constp.tile([128, W], f32, name="relc")
            nc.scalar.dma_start(out=iota[:], in_=iota_d[:])
            nc.scalar.dma_start(out=relc[:], in_=relc_d[:])
            # the whole compact feature slab (3.8 MB) is preloaded once on
            # the SWDGE ring: ZERO steady-state in-DMAs, so the out queue
            # owns the full SDMA port bandwidth. B rows land at partition
            # 64 (block-copy sources must start 32-aligned).
            big = constp.tile([64 + RP, W, C], f16, name="big")
            nc.gpsimd.dma_start(out=big[0:RP], in_=lhst_d[0:RP])
            nc.gpsimd.dma_start(out=big[64:64 + RP], in_=lhst_d[RP:2 * RP])
            # persistent ping-pong stationary tiles; zero regions (pad rows,
            # off-diagonal col blocks) are memset once and never rewritten.
            # lt1's memset is deferred into chunk 0's slot so the startup
            # chain to the first matmul stays short.
            lts = [lhstp.tile([128, CH, 128], f16, name=f"lt{pp}",
                              tag=f"lt{pp}") for pp in range(2)]
            nc.vector.memset(lts[0][:], 0.0)
            for pair in range(PAIRS):
                for ch in range(NCHUNKS):
                    g0 = pair * WPQ + ch * CH
                    ci = pair * NCHUNKS + ch
                    lt = lts[ci % 2]
                    if ci == 0:
                        # fast DVE copy for chunk 0: first matmul can
                        # start ~7us in instead of waiting a 7us Q7 copy
                        nc.vector.tensor_copy(out=lt[0:RP, :, 0:C],
                                              in_=big[0:RP, g0:g0 + CH, :])
                    else:
                        nc.gpsimd.tensor_copy(out=lt[0:RP, :, 0:C],
                                              in_=big[0:RP, g0:g0 + CH, :])
                    nc.vector.tensor_copy(out=lt[64:64 + RP, :, C:128],
                                          in_=big[64:64 + RP, g0:g0 + CH, :])
                    if ci == 0:
                        nc.gpsimd.memset(lts[1][:], 0.0)
                    st = stagep.tile([128, CH, NW], f32, name="st")
                    sub0 = 0
                    ngrp = (CH + GRP - 1) // GRP
                    for grp in range(ngrp):
                        j0 = grp * GRP
                        n = min(GRP, CH - j0)
                        acc = psump.tile([128, GRP, 512], f32, name="acc")
                        for par in range(n):
                            w = j0 + par
                            mask = maskp.tile([128, NW], f16, name="mask")
                            nc.vector.tensor_scalar(
                                out=mask[:],
                                in0=iota[:],
                                scalar1=relc[:, g0 + w:g0 + w + 1],
                                scalar2=None,
                                op0=mybir.AluOpType.is_equal,
                            )
                            nc.tensor.matmul(acc[:, par, 0:NW], lt[:, w, :],
                                             mask[:], start=True, stop=True)
                        nc.scalar.copy(st[:, j0:j0 + n, :], acc[:, 0:n, 0:NW])
                        # chunk 0 only: sub-chunk stores so the out queue
                        # starts ~8 windows in; later chunks use one big
                        # store (53KB descriptors run at the 27GB/s
                        # per-engine port cap; quarter-size ones pay ~12%
                        # more engine-busy)
                        if pair == 0 and ch == 0 and (grp % 2 == 1
                                                      or grp == ngrp - 1):
                            hi = j0 + n
                            nc.sync.dma_start(
                                out=out_d[pair, ch, :, sub0:hi, :],
                                in_=st[:, sub0:hi, :])
                            sub0 = hi
                    if not (pair == 0 and ch == 0):
                        nc.sync.dma_start(out=out_d[pair, ch], in_=st[:])
    nc.compile()
    return nc


def _pack(inputs):
    lhst = np.zeros((NCORES, 2 * RP, W, C), np.float16)
    iota = np.broadcast_to(np.arange(NW, dtype=np.float16),
                           (128, NW)).copy()
    relc_a = np.full((NCORES, 128, W), -1.0, np.float32)

    for bin_i in range(NBINS):
        feats = np.asarray(inputs[f"pillar_features_bin_{bin_i}"],
                           np.float32).astype(np.float16)
        coords = np.asarray(inputs[f"voxel_coords_bin_{bin_i}"])
        cb = np.asarray(coords[:, 0], np.int64)
        cy = np.asarray(coords[:, 2], np.int64)
        cx = np.asarray(coords[:, 3], np.int64)
        for b in range(B):
            rows_b = np.nonzero(cb == b)[0]
            y_b, x_b = cy[rows_b], cx[rows_b]
            for yq in range(4):
                q = bin_i * 16 + b * 4 + yq
                core, j = divmod(q, QPC)
                pair, half = divmod(j, 2)
                sel = (y_b >= YQ * yq) & (y_b < YQ * (yq + 1))
                rows = rows_b[sel]
                wv = y_b[sel] - YQ * yq
                rel = x_b[sel]
                order = np.argsort(wv, kind="stable")
                rows, wv, rel = rows[order], wv[order], rel[order]
                cnt = np.bincount(wv, minlength=WPQ)
                if cnt.max() > RP:
                    raise OverflowError(int(cnt.max()))
                off = np.concatenate([[0], np.cumsum(cnt)[:-1]])
                slot = np.arange(len(rows)) - off[wv]
                wins = pair * WPQ + wv
                lhst[core, half * RP + slot, wins, :] = feats[rows]
                relc_a[core, half * 64 + slot, wins] = rel
    return [{"lhst": lhst[c], "iotat": iota, "relc": relc_a[c]}
            for c in range(NCORES)]


def _run(inputs, trace=False):
    if "nc" not in _cache:
        _cache["nc"] = _build()
    nc = _cache["nc"]
    in_maps = _pack(inputs)
    res = run_bass_kernel_spmd(nc, in_maps, core_ids=list(range(NCORES)),
                               trace=trace)
    outs = [np.zeros((B, C, NY, NX), np.float32) for _ in range(NBINS)]
    for q in range(NQ):
        bin_i, rem = divmod(q, 16)
        b, yq = divmod(rem, 4)
        core, j = divmod(q, QPC)
        pair, half = divmod(j, 2)
        # [NCHUNKS, 64, CH, NW] -> [64, QCELLS] -> [64, YQ, NX]
        blk = res.results[core]["out"][pair, :, half * C:(half + 1) * C]
        outs[bin_i][b, :, YQ * yq:YQ * (yq + 1), :] = (
            blk.transpose(1, 0, 2, 3).reshape(C, YQ, NX))
    return tuple(outs), res


def kernel(**inputs):
    out, _ = _run(inputs)
    return out


def kernel_traced(**inputs):
    """Like kernel() but also returns BassKernelResults (for test.py)."""
    return _run(inputs, trace=True)


# revision 17
# speedup vs baseline: 1.1242x; 1.0740x over previous
"""PointPillarScatter (intersweep, 3 bins) Trainium2 Bass kernel.

Problem: for each of 3 bins, scatter 64000 pillar rows [64 feats] into a
[B=4, C=64, NY=496, NX=432] canvas at (b, :, y, x); empty cells are zero.

Strategy (8 NeuronCores, SPMD):
  - The output is 658 MB, ~92.5% zeros -> write-bandwidth bound. Dense
    output tiles (zeros included) are generated on-chip and stored with
    large contiguous DMAs; the compute stages are sized to fit under the
    ~19 us per-chunk out-DMA.
  - Host-side (cheap numpy): shard the 12 (bin, b) canvases into 48
    quarter-canvases of 124 y-rows; 6 per core, processed as 3 pairs.
    A window is ONE canvas row (432 cells); a row's pillars (max 36) go
    into RP=40 padded slots.
  - Per window one fp16 matmul places the pillar features:
      out[128, 432] = lhsT[128, 128].T @ onehot[128, 432]
    K rows hold [2 quarters x 40 slots] (A at 0:40, B at 64:104); all
    other K rows stay zero so the matmul runs with K=128 (full PE rate;
    K<128 runs half-rate). Features ship as a single fp16 term
    (rel err ~3e-4, well under the 2e-2 gate) -- a third the traffic of
    an exact 3-term bf16 split.
    onehot[k, c] = (x[k] == c): one DVE tensor_scalar is_equal of an
    fp16 iota row against per-partition relc scalars (432 < 2048 so
    fp16 holds the indices exactly; bf16 would not past 256).
  - lhsT is block-"diagonal" over quarters (A in M-cols 0:64, B in
    64:128). The compact feature slab is DMA'd contiguously (one ~4KB
    descriptor per partition; a strided DMA into lt shatters into 128B
    descriptors) and placed into the persistent pre-zeroed lt tiles by
    one GpSimd + one DVE block copy (partition starts must be 32-aligned).
  - 4 windows share one 4-bank PSUM tile; one wide ACT copy moves
    [128, 4, 432] to SBUF staging; one [128 x 53568 B] contiguous DMA
    per 31-window chunk writes out. Host de-interleaves the quarters.
  - in-DMAs ride the GpSimd SWDGE ring: they never queue behind the
    ~16 us out-DMAs on the sync-engine HWDGE ring (FIFO per ring), and
    their descriptor generation stays off the busy ACT engine.
"""

import numpy as np

import concourse.bass as bass
import concourse.tile as tile
from concourse import bacc, mybir
from concourse.bass_utils import run_bass_kernel_spmd

# Problem geometry (hardcoded; kernel.py must be self-contained).
B = 4
C = 64
NX = 432
NY = 496
NBINS = 3
NCORES = 8

NQ = NBINS * B * 4  # 48 quarter-canvases
YQ = NY // 4  # 124 y-rows per quarter
QPC = NQ // NCORES  # 6 quarters per core
PAIRS = QPC // 2  # 3 pairs per core
NW = NX  # one canvas row per window
WPQ = YQ  # 124 windows per quarter-pair
W = PAIRS * WPQ  # 372 windows per core
CH = 31  # windows per staging chunk
NCHUNKS = WPQ // CH  # 4 chunks per pair
GRP = 4  # windows per PSUM tile / ACT copy
RP = 40  # padded pillar slots per window per quarter (max count is 36)

_cache = {}


def _build():
    nc = bacc.Bacc(trn_type="TRN2")
    f16 = mybir.dt.float16
    f32 = mybir.dt.float32
    lhst_d = nc.dram_tensor("lhst", [2 * RP, W, C], f16,
                            kind="ExternalInput")
    iota_d = nc.dram_tensor("iotat", [128, NW], f16, kind="ExternalInput")
    relc_d = nc.dram_tensor("relc", [128, W], f32, kind="ExternalInput")
    out_d = nc.dram_tensor("out", [PAIRS, NCHUNKS, 128, CH, NW], f32,
                           kind="ExternalOutput")

    with tile.TileContext(nc) as tc:
        with (
            tc.tile_pool(name="const", bufs=1) as constp,
            tc.tile_pool(name="lhstp", bufs=1) as lhstp,
            tc.tile_pool(name="maskp", bufs=6) as maskp,
            tc.tile_pool(name="stage", bufs=2) as stagep,
            tc.tile_pool(name="psum", bufs=2, space=bass.MemorySpace.PSUM) as psump,
        ):
            iota = constp.tile([128, NW], f16, name="iota")
            relc = constp.tile([128, W], f32, name="relc")
            nc.scalar.dma_start(out=iota[:], in_=iota_d[:])
            nc.scalar.dma_start(out=relc[:], in_=relc_d[:])
            # the whole compact feature slab (3.8 MB) is preloaded once on
            # the SWDGE ring: ZERO steady-state in-DMAs, so the out queue
            # owns the full SDMA port bandwidth. B rows land at partition
            # 64 (block-copy sources must start 32-aligned).
            big = constp.tile([64 + RP, W, C], f16, name="big")
            # chunk 0's slice first (~0.3MB) so its block copies start ~13us
            # in; the rest of the slab streams behind it
            nc.gpsimd.dma_start(out=big[0:RP, 0:CH], in_=lhst_d[0:RP, 0:CH])
            nc.gpsimd.dma_start(out=big[64:64 + RP, 0:CH],
                                in_=lhst_d[RP:2 * RP, 0:CH])
            nc.gpsimd.dma_start(out=big[0:RP, CH:W], in_=lhst_d[0:RP, CH:W])
            nc.gpsimd.dma_start(out=big[64:64 + RP, CH:W],
                                in_=lhst_d[RP:2 * RP, CH:W])
            # persistent ping-pong stationary tiles; zero regions (pad rows,
            # off-diagonal col blocks) are memset once and never rewritten.
            # lt1's memset is deferred into chunk 0's slot so the startup
            # chain to the first matmul stays short.
            lts = [lhstp.tile([128, CH, 128], f16, name=f"lt{pp}",
                              tag=f"lt{pp}") for pp in range(2)]
            nc.vector.memset(lts[0][:], 0.0)
            for pair in range(PAIRS):
                for ch in range(NCHUNKS):
                    g0 = pair * WPQ + ch * CH
                    ci = pair * NCHUNKS + ch
                    lt = lts[ci % 2]
                    if ci == 0:
                        # fast DVE copy for chunk 0: first matmul can
                        # start ~7us in instead of waiting a 7us Q7 copy
                        nc.vector.tensor_copy(out=lt[0:RP, :, 0:C],
                                              in_=big[0:RP, g0:g0 + CH, :])
                    else:
                        nc.gpsimd.tensor_copy(out=lt[0:RP, :, 0:C],
                                              in_=big[0:RP, g0:g0 + CH, :])
                    nc.vector.tensor_copy(out=lt[64:64 + RP, :, C:128],
                                          in_=big[64:64 + RP, g0:g0 + CH, :])
                    if ci == 0:
                        nc.gpsimd.memset(lts[1][:], 0.0)
                    st = stagep.tile([128, CH, NW], f32, name="st")
                    sub0 = 0
                    ngrp = (CH + GRP - 1) // GRP
                    for grp in range(ngrp):
                        j0 = grp * GRP
                        n = min(GRP, CH - j0)
                        acc = psump.tile([128, GRP, 512], f32, name="acc")
                        for par in range(n):
                            w = j0 + par
                            mask = maskp.tile([128, NW], f16, name="mask")
                            nc.vector.tensor_scalar(
                                out=mask[:],
                                in0=iota[:],
                                scalar1=relc[:, g0 + w:g0 + w + 1],
                                scalar2=None,
                                op0=mybir.AluOpType.is_equal,
                            )
                            nc.tensor.matmul(acc[:, par, 0:NW], lt[:, w, :],
                                             mask[:], start=True, stop=True)
                        nc.scalar.copy(st[:, j0:j0 + n, :], acc[:, 0:n, 0:NW])
                        # half-chunk stores (27.6KB descriptors keep the
                        # 27GB/s per-engine rate): the store starts mid-way
                        # through the ACT copy chain instead of after it,
                        # and the staging WAR releases earlier
                        if grp == 3 or grp == ngrp - 1:
                            hi = j0 + n
                            nc.sync.dma_start(
                                out=out_d[pair, ch, :, sub0:hi, :],
                                in_=st[:, sub0:hi, :])
                            sub0 = hi
    nc.compile()
    return nc


def _pack(inputs):
    lhst = np.zeros((NCORES, 2 * RP, W, C), np.float16)
    iota = np.broadcast_to(np.arange(NW, dtype=np.float16),
                           (128, NW)).copy()
    relc_a = np.full((NCORES, 128, W), -1.0, np.float32)

    for bin_i in range(NBINS):
        feats = np.asarray(inputs[f"pillar_features_bin_{bin_i}"],
                           np.float32).astype(np.float16)
        coords = np.asarray(inputs[f"voxel_coords_bin_{bin_i}"])
        cb = np.asarray(coords[:, 0], np.int64)
        cy = np.asarray(coords[:, 2], np.int64)
        cx = np.asarray(coords[:, 3], np.int64)
        for b in range(B):
            rows_b = np.nonzero(cb == b)[0]
            y_b, x_b = cy[rows_b], cx[rows_b]
            for yq in range(4):
                q = bin_i * 16 + b * 4 + yq
                core, j = divmod(q, QPC)
                pair, half = divmod(j, 2)
                sel = (y_b >= YQ * yq) & (y_b < YQ * (yq + 1))
                rows = rows_b[sel]
                wv = y_b[sel] - YQ * yq
                rel = x_b[sel]
                order = np.argsort(wv, kind="stable")
                rows, wv, rel = rows[order], wv[order], rel[order]
                cnt = np.bincount(wv, minlength=WPQ)
                if cnt.max() > RP:
                    raise OverflowError(int(cnt.max()))
                off = np.concatenate([[0], np.cumsum(cnt)[:-1]])
                slot = np.arange(len(rows)) - off[wv]
                wins = pair * WPQ + wv
                lhst[core, half * RP + slot, wins, :] = feats[rows]
                relc_a[core, half * 64 + slot, wins] = rel
    return [{"lhst": lhst[c], "iotat": iota, "relc": relc_a[c]}
            for c in range(NCORES)]


def _run(inputs, trace=False):
    if "nc" not in _cache:
        _cache["nc"] = _build()
    nc = _cache["nc"]
    in_maps = _pack(inputs)
    res = run_bass_kernel_spmd(nc, in_maps, core_ids=list(range(NCORES)),
                               trace=trace)
    outs = [np.zeros((B, C, NY, NX), np.float32) for _ in range(NBINS)]
    for q in range(NQ):
        bin_i, rem = divmod(q, 16)
        b, yq = divmod(rem, 4)
        core, j = divmod(q, QPC)
        pair, half = divmod(j, 2)
        # [NCHUNKS, 64, CH, NW] -> [64, QCELLS] -> [64, YQ, NX]
        blk = res.results[core]["out"][pair, :, half * C:(half + 1) * C]
        outs[bin_i][b, :, YQ * yq:YQ * (yq + 1), :] = (
            blk.transpose(1, 0, 2, 3).reshape(C, YQ, NX))
    return tuple(outs), res


def kernel(**inputs):
    out, _ = _run(inputs)
    return out


def kernel_traced(**inputs):
    """Like kernel() but also returns BassKernelResults (for test.py)."""
    return _run(inputs, trace=True)
